# revision 1
# baseline (speedup 1.0000x reference)
"""Deformable conv Trainium2 kernel: builder + host prep + numpy staged model.

Per core: NIMG=2 images (data-parallel over batch N=16 across 8 cores).

Pipeline per image:
  A. offset conv on PE (bf16): 9 shifted-AP taps accumulate -> off [18,4096] f32
  B. off -> HBM -> wrap-read dy/dx as [128,288] (sample s = p*4096+l, part=s%128,
     chunk = s//128 = p*32 + l//128)
  C. index math on DVE (fp32): corner rows/cols, validity, 4 bilinear weights
     (bf16), int16 pair-gather indices jt/jb
  D. jt/jb -> HBM -> wrap-16 read-back [128,2304] replicated idx tensors
  E. per (tap, half): dma_gather (bf16, 512B pair descriptors) top+bottom rows
  F. blend on DVE: G*omega broadcast-mult + pair adds -> sT [s-part, c] bf16
  G. PE transpose 128-blocks -> s_all [c, tap, l] bf16
  H. main conv on PE: 9-tap matmul accumulate + BN/SiLU on ACT -> y f32
"""

import numpy as np
import ml_dtypes

import concourse.bass as bass
import concourse.mybir as mybir
import concourse.tile as tile

F32 = mybir.dt.float32
BF16 = mybir.dt.bfloat16
I16 = mybir.dt.int16

NIMG = 2
H = W = 64
HW = H * W          # 4096
P = 9               # taps
NS = P * HW         # 36864 samples per image
NCHUNK = NS // 128  # 288
NF = NCHUNK * 8     # 2304 idx free size (wrapped-16)

bf = ml_dtypes.bfloat16


# ----------------------------------------------------------------- host prep
def host_consts():
    part = np.arange(128)[:, None]          # [128,1]
    chunk = np.arange(NCHUNK)[None, :]      # [1,288]
    p = chunk // 32                          # tap
    l = (chunk % 32) * 128 + part            # [128,288]
    ho, wo = l // 64, l % 64
    ky, kx = p // 3, p % 3
    ay8 = (ky + ho - 1 + 8).astype(np.float32)
    ax8 = (kx + wo - 1 + 8).astype(np.float32)
    ident = np.eye(128, dtype=bf)
    return dict(ay8=ay8, ax8=ax8, ident=ident)


def host_prep_weights(offset_w, offset_b, deform_w, gamma, beta, rm, rv, eps=1e-5):
    # offset_w [18,128,3,3] -> woff [128 cin, 9 tap, 18]
    woff = np.transpose(offset_w, (1, 2, 3, 0)).reshape(128, 9, 18).astype(bf)
    offb = offset_b.reshape(18, 1).astype(np.float32)
    # deform_w [128 o,128 c,3,3] -> wdef [128 c, 9 tap, 128 o]
    wdef = np.transpose(deform_w, (1, 2, 3, 0)).reshape(128, 9, 128).astype(bf)
    scale = (gamma / np.sqrt(rv + eps)).astype(np.float32).reshape(128, 1)
    shift = (beta - rm * gamma / np.sqrt(rv + eps)).astype(np.float32).reshape(128, 1)
    return dict(woff=woff, offb=offb, wdef=wdef, bnscale=scale, bnshift=shift)


def host_prep_images(x2):
    """x2: [n,128,64,64] f32 -> xpad bf16 [n,128,4356], xpair bf16 [n,4097,256]."""
    n = x2.shape[0]
    xpad = np.zeros((n, 128, 66, 66), dtype=bf)
    xpad[:, :, 1:65, 1:65] = x2.astype(bf)
    xpad = xpad.reshape(n, 128, 66 * 66)

    xloc = np.transpose(x2, (0, 2, 3, 1)).reshape(n, HW, 128).astype(bf)
    xloc_p = np.zeros((n, HW + 2, 128), dtype=bf)
    xloc_p[:, 1:-1] = xloc
    xpair = np.concatenate([xloc_p[:, :-1], xloc_p[:, 1:]], axis=2)  # [n,4097,256]
    return xpad, np.ascontiguousarray(xpair)


def make_in_map(x2, offset_w, offset_b, deform_w, gamma, beta, rm, rv):
    con = host_consts()
    wts = host_prep_weights(offset_w, offset_b, deform_w, gamma, beta, rm, rv)
    xpad, xpair = host_prep_images(x2)
    return dict(
        xpad=np.ascontiguousarray(xpad), xpair=xpair,
        woff=np.ascontiguousarray(wts["woff"]), offb=wts["offb"],
        wdef=np.ascontiguousarray(wts["wdef"]),
        bnscale=wts["bnscale"], bnshift=wts["bnshift"],
        ay8=con["ay8"], ax8=con["ax8"], ident=con["ident"],
    )


def nid0(g, NG):
    return NG * g


# ------------------------------------------------------------------- builder
def build(nc, stage="full", nimg=NIMG):
    dt = nc.dram_tensor
    xpad_d = dt("xpad", [nimg, 128, 66 * 66], BF16, kind="ExternalInput")
    xpair_d = dt("xpair", [nimg, 4097, 256], BF16, kind="ExternalInput")
    woff_d = dt("woff", [128, 9, 18], BF16, kind="ExternalInput")
    offb_d = dt("offb", [18, 1], F32, kind="ExternalInput")
    wdef_d = dt("wdef", [128, 9, 128], BF16, kind="ExternalInput")
    bnscale_d = dt("bnscale", [128, 1], F32, kind="ExternalInput")
    bnshift_d = dt("bnshift", [128, 1], F32, kind="ExternalInput")
    ay8_d = dt("ay8", [128, NCHUNK], F32, kind="ExternalInput")
    ax8_d = dt("ax8", [128, NCHUNK], F32, kind="ExternalInput")
    ident_d = dt("ident", [128, 128], BF16, kind="ExternalInput")

    y_d = dt("y", [nimg, 128, HW], F32, kind="ExternalOutput")
    dbg = {}
    if stage == "off":
        dbg["off"] = dt("dbg_off", [nimg, 18, HW], F32, kind="ExternalOutput")
    if stage == "idx":
        dbg["omt"] = dt("dbg_omt", [nimg, 128, NCHUNK, 2], BF16, kind="ExternalOutput")
        dbg["omb"] = dt("dbg_omb", [nimg, 128, NCHUNK, 2], BF16, kind="ExternalOutput")
        dbg["idxt"] = dt("dbg_idxt", [nimg, 128, NF], I16, kind="ExternalOutput")
        dbg["idxb"] = dt("dbg_idxb", [nimg, 128, NF], I16, kind="ExternalOutput")
    if stage in ("gather", "blend"):
        dbg["st"] = dt("dbg_st", [nimg, 128, NS], BF16, kind="ExternalOutput")
    if stage == "gonly":
        dbg["gt"] = dt("dbg_gt", [nimg, 128, 8, 256], BF16, kind="ExternalOutput")

    ALU = mybir.AluOpType
    ACT = mybir.ActivationFunctionType

    with tile.TileContext(nc) as tc:
        with (
            tc.tile_pool(name="const", bufs=1) as cpool,
            tc.tile_pool(name="xin", bufs=1) as xpool,
            tc.tile_pool(name="offp", bufs=1) as offpool,
            tc.tile_pool(name="idxp", bufs=1) as idxpool,
            tc.tile_pool(name="gat", bufs=1) as gpool,
            tc.tile_pool(name="sall", bufs=1) as spool,
            tc.tile_pool(name="yout", bufs=2) as ypool,
            tc.tile_pool(name="psoff", bufs=2, space="PSUM") as psoff,
            tc.tile_pool(name="pstr", bufs=4, space="PSUM") as pstr,
            tc.tile_pool(name="psy", bufs=2, space="PSUM") as psy,
            tc.tile_pool(name="dram", bufs=2, space="DRAM") as dpool,
        ):
            C = {}
            for nm, d, shp, ddt in [
                ("woff", woff_d, [128, 9, 18], BF16),
                ("offb", offb_d, [18, 1], F32),
                ("wdef", wdef_d, [128, 9, 128], BF16),
                ("bnscale", bnscale_d, [128, 1], F32),
                ("bnshift", bnshift_d, [128, 1], F32),
                ("ay8", ay8_d, [128, NCHUNK], F32),
                ("ax8", ax8_d, [128, NCHUNK], F32),
                ("ident", ident_d, [128, 128], BF16),
            ]:
                s = cpool.tile(shp, ddt, name=nm, tag=nm)
                nc.sync.dma_start(s[:], d[:])
                C[nm] = s

            for img in range(nimg):
                _image(nc, tc, stage, img, dbg, xpad_d, xpair_d, y_d, C,
                       xpool, offpool, idxpool, gpool, spool, ypool,
                       psoff, pstr, psy, dpool, ALU, ACT)
    return nc


def _image(nc, tc, stage, img, dbg, xpad_d, xpair_d, y_d, C,
           xpool, offpool, idxpool, gpool, spool, ypool,
           psoff, pstr, psy, dpool, ALU, ACT):
    nq = getattr(nc, "num_swdge_queues", 1)
    # ---- A. offset conv
    xpad_s = xpool.tile([128, 66 * 66], BF16, name="xpad", tag="xpad")
    nc.sync.dma_start(xpad_s[:], xpad_d[img])
    xv = xpad_s[:].rearrange("c (h w) -> c h w", h=66)

    off_s = offpool.tile([18, HW], F32, name="off", tag="off")
    for q in range(8):
        ps = psoff.tile([18, 512], F32, name="offps", tag="offps")
        for p in range(9):
            ky, kx = p // 3, p % 3
            rhs = xv[:, 8 * q + ky: 8 * q + ky + 8, kx: kx + 64]
            nc.tensor.matmul(ps[:], C["woff"][:, p, :], rhs,
                             start=(p == 0), stop=(p == 8))
        nc.vector.tensor_scalar(
            off_s[:, 512 * q: 512 * (q + 1)], ps[:], C["offb"][:], None, ALU.add
        )
    if stage == "off":
        nc.sync.dma_start(dbg["off"][img], off_s[:])
        return

    # ---- B. wrap-read dy/dx
    offd = dpool.tile([18, HW], F32, name="offd", tag="offd")
    nc.sync.dma_start(offd[:], off_s[:])
    dy_s = idxpool.tile([128, NCHUNK], F32, name="dy", tag="dy")
    dx_s = idxpool.tile([128, NCHUNK], F32, name="dx", tag="dx")
    offd_t = offd[:].tensor
    for p in range(9):
        nc.sync.dma_start(
            dy_s[:, 32 * p: 32 * (p + 1)],
            bass.AP(offd_t, 2 * p * HW, [[1, 128], [128, 32]]),
        )
        nc.sync.dma_start(
            dx_s[:, 32 * p: 32 * (p + 1)],
            bass.AP(offd_t, (2 * p + 1) * HW, [[1, 128], [128, 32]]),
        )

    # ---- C. index math
    def t(tag):
        return idxpool.tile([128, NCHUNK], F32, name=tag, tag=tag)

    I32 = mybir.dt.int32
    py8 = t("py8"); nc.vector.tensor_tensor(py8[:], dy_s[:], C["ay8"][:], ALU.add)
    yi = idxpool.tile([128, NCHUNK], I32, name="i32y", tag="i32y")
    nc.vector.tensor_copy(yi[:], py8[:])
    yf = t("yf");  nc.vector.tensor_copy(yf[:], yi[:])
    ygt = t("ygt"); nc.vector.tensor_tensor(ygt[:], yf[:], py8[:], ALU.is_gt)
    y0 = t("y0");  nc.vector.tensor_tensor(y0[:], yf[:], ygt[:], ALU.subtract)
    wy = t("wy");  nc.vector.tensor_tensor(wy[:], py8[:], y0[:], ALU.subtract)
    yct = t("yct"); nc.vector.tensor_scalar(yct[:], y0[:], 8.0, 71.0, ALU.max, ALU.min)
    vt = t("sc0"); nc.vector.tensor_tensor(vt[:], y0[:], yct[:], ALU.is_equal)
    y1 = t("sc1"); nc.vector.tensor_scalar(y1[:], y0[:], 1.0, None, ALU.add)
    ycb = t("ycb"); nc.vector.tensor_scalar(ycb[:], y1[:], 8.0, 71.0, ALU.max, ALU.min)
    vb = t("sc2"); nc.vector.tensor_tensor(vb[:], y1[:], ycb[:], ALU.is_equal)

    px8 = t("px8"); nc.vector.tensor_tensor(px8[:], dx_s[:], C["ax8"][:], ALU.add)
    xi = idxpool.tile([128, NCHUNK], I32, name="i32x", tag="i32x")
    nc.vector.tensor_copy(xi[:], px8[:])
    xf = t("xf");  nc.vector.tensor_copy(xf[:], xi[:])
    xgt = t("xgt"); nc.vector.tensor_tensor(xgt[:], xf[:], px8[:], ALU.is_gt)
    x0 = t("sc3"); nc.vector.tensor_tensor(x0[:], xf[:], xgt[:], ALU.subtract)
    wx = t("wx");  nc.vector.tensor_tensor(wx[:], px8[:], x0[:], ALU.subtract)
    xc = t("xc");  nc.vector.tensor_scalar(xc[:], x0[:], 7.0, 71.0, ALU.max, ALU.min)
    cl = t("sc4"); nc.vector.tensor_scalar(cl[:], x0[:], 8.0, 71.0, ALU.max, ALU.min)
    vxl = t("sc5"); nc.vector.tensor_tensor(vxl[:], x0[:], cl[:], ALU.is_equal)
    cr = t("sc6"); nc.vector.tensor_scalar(cr[:], x0[:], 7.0, 70.0, ALU.max, ALU.min)
    vxr = t("sc7"); nc.vector.tensor_tensor(vxr[:], x0[:], cr[:], ALU.is_equal)

    w1y = t("sc8"); nc.vector.tensor_scalar(w1y[:], wy[:], -1.0, 1.0, ALU.mult, ALU.add)
    w1x = t("sc9"); nc.vector.tensor_scalar(w1x[:], wx[:], -1.0, 1.0, ALU.mult, ALU.add)
    q0 = t("q0");  nc.vector.tensor_tensor(q0[:], w1y[:], vt[:], ALU.mult)
    q1 = t("q1");  nc.vector.tensor_tensor(q1[:], wy[:], vb[:], ALU.mult)
    r0 = t("r0");  nc.vector.tensor_tensor(r0[:], w1x[:], vxl[:], ALU.mult)
    r1 = t("r1");  nc.vector.tensor_tensor(r1[:], wx[:], vxr[:], ALU.mult)

    omt = idxpool.tile([128, NCHUNK, 2], BF16, name="omt", tag="omt")
    omb = idxpool.tile([128, NCHUNK, 2], BF16, name="omb", tag="omb")
    nc.vector.tensor_tensor(omt[:, :, 0], q0[:], r0[:], ALU.mult)
    nc.vector.tensor_tensor(omt[:, :, 1], q0[:], r1[:], ALU.mult)
    nc.vector.tensor_tensor(omb[:, :, 0], q1[:], r0[:], ALU.mult)
    nc.vector.tensor_tensor(omb[:, :, 1], q1[:], r1[:], ALU.mult)

    jtf = t("jtf")
    nc.vector.scalar_tensor_tensor(jtf[:], yct[:], 64.0, xc[:], ALU.mult, ALU.add)
    nc.vector.tensor_scalar(jtf[:], jtf[:], -519.0, None, ALU.add)
    jbf = t("jbf")
    nc.vector.scalar_tensor_tensor(jbf[:], ycb[:], 64.0, xc[:], ALU.mult, ALU.add)
    nc.vector.tensor_scalar(jbf[:], jbf[:], -519.0, None, ALU.add)
    jt16 = idxpool.tile([128, NCHUNK], I16, name="jt16", tag="jt16")
    jb16 = idxpool.tile([128, NCHUNK], I16, name="jb16", tag="jb16")
    nc.vector.tensor_copy(jt16[:], jtf[:])
    nc.vector.tensor_copy(jb16[:], jbf[:])

    # ---- D. wrap-16 replicated idx tensors
    jd = dpool.tile([2, 128, NCHUNK], I16, name="jd", tag="jd")
    nc.sync.dma_start(jd[0], jt16[:])
    nc.sync.dma_start(jd[1], jb16[:])
    idxt = idxpool.tile([128, NF], I16, name="idxt", tag="idxt")
    idxb = idxpool.tile([128, NF], I16, name="idxb", tag="idxb")
    jd_t = jd[:].tensor
    # dst column f = 8*chunk + g holds j[p16 + 16g, chunk], replicated over
    # the 8 16-partition groups. One strided DMA per g.
    for g in range(8):
        src = [[0, 8], [NCHUNK, 16], [1, NCHUNK]]
        nc.sync.dma_start(
            bass.AP(idxt[:].tensor, g, [[NF, 128], [8, NCHUNK]]),
            bass.AP(jd_t, 16 * g * NCHUNK, [list(r) for r in src]),
        )
        nc.sync.dma_start(
            bass.AP(idxb[:].tensor, g, [[NF, 128], [8, NCHUNK]]),
            bass.AP(jd_t, 128 * NCHUNK + 16 * g * NCHUNK, [list(r) for r in src]),
        )

    if stage == "idx":
        nc.sync.dma_start(dbg["omt"][img], omt[:])
        nc.sync.dma_start(dbg["omb"][img], omb[:])
        nc.sync.dma_start(dbg["idxt"][img], idxt[:])
        nc.sync.dma_start(dbg["idxb"][img], idxb[:])
        return

    # ---- E/F/G. gather + blend + transpose, per (tap, half of 16 chunks)
    s_all = spool.tile([128, 9, HW], BF16, name="sall", tag="sall")
    NG = 8                                    # chunks per gather group
    for p in range(9):
        for h in range(32 // NG):
            g = (32 // NG) * p + h           # group index
            c0 = 32 * p + NG * h             # first global chunk of group
            nid = NG * 128
            gt = gpool.tile([128, NG, 256], BF16, name="gt", tag="gt")
            gb = gpool.tile([128, NG, 256], BF16, name="gb", tag="gb")
            f0 = 8 * c0
            nc.gpsimd.dma_gather(
                gt[:], xpair_d[img], idxt[:, f0: f0 + NG * 8],
                num_idxs=nid, num_idxs_reg=nid, elem_size=256,
                queue_num=(2 * g) % nq,
            )
            nc.gpsimd.dma_gather(
                gb[:], xpair_d[img], idxb[:, f0: f0 + NG * 8],
                num_idxs=nid, num_idxs_reg=nid, elem_size=256,
                queue_num=(2 * g + 1) % nq,
            )
            if stage == "gonly":
                if p == 0 and h == 0:
                    nc.sync.dma_start(dbg["gt"][img], gt[:])
                continue
            m0 = gpool.tile([128, NG, 128], BF16, name="m0", tag="m0")
            m1 = gpool.tile([128, NG, 128], BF16, name="m1", tag="m1")
            m2 = gpool.tile([128, NG, 128], BF16, name="m2", tag="m2")
            m3 = gpool.tile([128, NG, 128], BF16, name="m3", tag="m3")
            sl = slice(c0, c0 + NG)
            bc = [128, NG, 128]
            nc.vector.tensor_tensor(m0[:], gt[:, :, 0:128], omt[:, sl, 0].unsqueeze(2).broadcast_to(bc), ALU.mult)
            nc.vector.tensor_tensor(m1[:], gt[:, :, 128:256], omt[:, sl, 1].unsqueeze(2).broadcast_to(bc), ALU.mult)
            nc.vector.tensor_tensor(m2[:], gb[:, :, 0:128], omb[:, sl, 0].unsqueeze(2).broadcast_to(bc), ALU.mult)
            nc.vector.tensor_tensor(m3[:], gb[:, :, 128:256], omb[:, sl, 1].unsqueeze(2).broadcast_to(bc), ALU.mult)
            s1 = gpool.tile([128, NG, 128], BF16, name="s1", tag="s1")
            s2 = gpool.tile([128, NG, 128], BF16, name="s2", tag="s2")
            st = gpool.tile([128, NG, 128], BF16, name="stt", tag="stt")
            nc.vector.tensor_tensor(s1[:], m0[:], m1[:], ALU.add)
            nc.vector.tensor_tensor(s2[:], m2[:], m3[:], ALU.add)
            nc.vector.tensor_tensor(st[:], s1[:], s2[:], ALU.add)
            if stage in ("gather", "blend"):
                nc.sync.dma_start(
                    dbg["st"][img][:, 128 * nid0(g, NG): 128 * nid0(g, NG) + nid],
                    st[:].rearrange("a b c -> a (b c)"),
                )
            if stage == "blend":
                continue
            # transpose NG blocks [s,c] -> [c,s]
            for tb in range(NG):
                tps = pstr.tile([128, 128], BF16, name="tps", tag="tps")
                nc.tensor.transpose(tps[:], st[:, tb, :], C["ident"][:])
                lblk = NG * h + tb           # l-block within tap (0..31)
                dst = s_all[:, p, 128 * lblk: 128 * (lblk + 1)]
                if tb % 2 == 0:
                    nc.scalar.copy(dst, tps[:])
                else:
                    nc.vector.tensor_copy(dst, tps[:])
    if stage in ("gather", "blend", "gonly"):
        return

    # ---- H. main conv + BN + SiLU
    for q in range(8):
        ps = psy.tile([128, 512], F32, name="yps", tag="yps")
        for p in range(9):
            rhs = s_all[:, p, 512 * q: 512 * (q + 1)]
            nc.tensor.matmul(ps[:], C["wdef"][:, p, :], rhs,
                             start=(p == 0), stop=(p == 8))
        ysb = ypool.tile([128, 512], F32, name="ysb", tag="ysb")
        zt = ypool.tile([128, 512], F32, name="zt", tag="zt")
        sg = ypool.tile([128, 512], F32, name="sg", tag="sg")
        nc.scalar.activation(zt[:], ps[:], ACT.Identity,
                             bias=C["bnshift"][:], scale=C["bnscale"][:])
        nc.scalar.activation(sg[:], ps[:], ACT.Sigmoid,
                             bias=C["bnshift"][:], scale=C["bnscale"][:])
        nc.vector.tensor_tensor(ysb[:], zt[:], sg[:], ALU.mult)
        nc.sync.dma_start(y_d[img][:, 512 * q: 512 * (q + 1)], ysb[:])


# ------------------------------------------------------------ numpy model
def numpy_stages(x2, offset_w, offset_b, deform_w, gamma, beta, rm, rv):
    n = x2.shape[0]
    con = host_consts()
    wts = host_prep_weights(offset_w, offset_b, deform_w, gamma, beta, rm, rv)
    xpad, xpair = host_prep_images(x2)
    out = {}
    xp = xpad.reshape(n, 128, 66, 66).astype(np.float32)
    woff = wts["woff"].astype(np.float32)           # [128,9,18]
    off = np.zeros((n, 18, 64, 64), np.float32)
    for p in range(9):
        ky, kx = p // 3, p % 3
        patch = xp[:, :, ky: ky + 64, kx: kx + 64]
        off += np.einsum("nchw,co->nohw", patch, woff[:, p, :])
    off += wts["offb"].reshape(1, 18, 1, 1)
    out["off"] = off.reshape(n, 18, HW)

    part = np.arange(128)[:, None]
    chunk = np.arange(NCHUNK)[None, :]
    p_ = np.broadcast_to(chunk // 32, (128, NCHUNK))
    l_ = (chunk % 32) * 128 + part
    dy = out["off"][:, 2 * p_, l_]                   # [n,128,288]
    dx = out["off"][:, 2 * p_ + 1, l_]
    py8 = dy + con["ay8"]
    px8 = dx + con["ax8"]
    wy = np.mod(py8, 1.0); y0 = py8 - wy
    wx = np.mod(px8, 1.0); x0 = px8 - wx
    yct = np.clip(y0, 8, 71); vt = (y0 == yct)
    y1 = y0 + 1; ycb = np.clip(y1, 8, 71); vb = (y1 == ycb)
    xc = np.clip(x0, 7, 71)
    vxl = x0 == np.clip(x0, 8, 71)
    vxr = x0 == np.clip(x0, 7, 70)
    q0 = (1 - wy) * vt; q1 = wy * vb
    r0 = (1 - wx) * vxl; r1 = wx * vxr
    omt = np.stack([q0 * r0, q0 * r1], axis=-1).astype(bf)
    omb = np.stack([q1 * r0, q1 * r1], axis=-1).astype(bf)
    jt = (yct * 64 + xc - 519).astype(np.int16)
    jb = (ycb * 64 + xc - 519).astype(np.int16)
    out.update(omt=omt, omb=omb, jt=jt, jb=jb)

    st = np.zeros((n, 128, NS), dtype=bf)
    for i in range(n):
        gt = xpair[i][jt[i]]            # [128,288,256] bf16
        gb = xpair[i][jb[i]]
        omt_b = np.repeat(omt[i][:, :, :, None], 128, axis=3).reshape(128, NCHUNK, 256)
        omb_b = np.repeat(omb[i][:, :, :, None], 128, axis=3).reshape(128, NCHUNK, 256)
        a = (gt * omt_b).astype(bf)
        b = (gb * omb_b).astype(bf)
        s1 = (a[:, :, 0:128] + a[:, :, 128:256]).astype(bf)
        s2 = (b[:, :, 0:128] + b[:, :, 128:256]).astype(bf)
        st[i] = (s1 + s2).astype(bf).reshape(128, NS)
    out["st"] = st

    y = np.zeros((n, 128, HW), np.float32)
    wdef = wts["wdef"].astype(np.float32)           # [128,9,128]
    for i in range(n):
        s_sm = st[i].astype(np.float32).reshape(128, NCHUNK, 128)
        s_cs = np.transpose(s_sm, (2, 1, 0)).reshape(128, NS)  # [c, s]
        for p in range(9):
            sl = s_cs[:, p * HW: (p + 1) * HW]
            y[i] += np.einsum("cl,co->ol", sl, wdef[:, p, :])
    scale = wts["bnscale"].reshape(1, 128, 1)
    shift = wts["bnshift"].reshape(1, 128, 1)
    ybn = y * scale + shift
    out["y"] = ybn * (1.0 / (1.0 + np.exp(-ybn)))
    return out


# ============================= tilefix =============================
from concourse.vector_clock import ScopedClock



from concourse.vector_clock import ScopedClock

_MAX_WAITS = 1


def _patched_drain_and_barrier(self, tick_clock, wait_clock):
    nc = self.nc
    collector = nc.sync.nop(nofuse=True)
    wait_clock.add_sem_waits(
        collector.ins, ScopedClock({None: tick_clock.global_clock})
    )
    si = collector.ins.sync_info
    waits = list(si.on_wait or []) if si is not None else []
    if si is not None:
        si.on_wait = waits[:_MAX_WAITS]
    for i in range(_MAX_WAITS, len(waits), _MAX_WAITS):
        n = nc.sync.nop(nofuse=True)
        nsi = n.ins.sync_info
        if nsi is None:
            n.ins.sync_info = type(si)(on_wait=waits[i : i + _MAX_WAITS], on_update=[])
        else:
            nsi.on_wait = waits[i : i + _MAX_WAITS]
    nc.sync.drain()

    nc.all_engine_barrier()
    assert self.sems is not None
    popped = nc._tile_sem_poison_stack.pop()
    assert popped is self._sem_poison
    nc.clear_and_free_semaphores(list(self.sems.allocated().values()))
    nc.all_engine_barrier()


def _apply_tilefix():
    tile.TileContext._drain_and_barrier = _patched_drain_and_barrier


# ===================================================================== kernel
_NC_CACHE = {}


def _build_nc():
    if "nc" not in _NC_CACHE:
        _apply_tilefix()
        import concourse.bacc as bacc
        nc = bacc.Bacc(None)
        build(nc, stage="full", nimg=NIMG)
        nc.compile()
        _NC_CACHE["nc"] = nc
    return _NC_CACHE["nc"]


def kernel(x, offset_w, offset_b, deform_w, gamma, beta, running_mean,
           running_var):
    from concourse.bass_utils import run_bass_kernel_spmd

    x = np.asarray(x); offset_w = np.asarray(offset_w)
    offset_b = np.asarray(offset_b); deform_w = np.asarray(deform_w)
    gamma = np.asarray(gamma); beta = np.asarray(beta)
    rm = np.asarray(running_mean); rv = np.asarray(running_var)

    nc = _build_nc()
    n_cores = 8
    per = x.shape[0] // n_cores  # 2
    in_maps = []
    for i in range(n_cores):
        in_maps.append(make_in_map(
            x[per * i: per * (i + 1)], offset_w, offset_b, deform_w,
            gamma, beta, rm, rv))
    res = run_bass_kernel_spmd(nc, in_maps, list(range(n_cores)))
    out = np.concatenate([r["y"] for r in res.results], axis=0)
    return out.reshape(16, 128, 64, 64).astype(np.float32)



# revision 2
# speedup vs baseline: 1.2431x; 1.2431x over previous
"""Deformable conv Trainium2 kernel v2: quad-table gather, restructured pipeline.

Per core: NIMG=2 images (data-parallel over batch N=16 across 8 cores).

Pipeline per image:
  A. offset conv on PE (bf16): 9 shifted-AP taps accumulate -> off [18,4096] f32
  B. off -> HBM -> wrap-read dy/dx as [128,288] (sample s = p*4096+l, part=s%128,
     chunk = s//128 = p*32 + l//128)
  C. index math on DVE (fp32): floor, frac, quad-table index j (zero-padded
     space, no validity masks needed), 4 bilinear weights (bf16)
  D. j16 -> HBM -> wrap-16 read-back [128,2304] replicated idx tensor
  E. per group of 8 chunks: ONE dma_gather from xquad [5041,512] bf16
     (each 1KB row carries all 4 corners x 128 channels), 4 SWDGE queues,
     6-deep buffer rotation
  F. blend on DVE: 4 products + 3 adds -> st [s-part, c] bf16
  G. PE transpose 128-blocks packed 4/psum-bank -> ACT copy -> s_all [c,tap,l]
  H. main conv on PE: 9-tap matmul accumulate + BN/SiLU on ACT -> y f32
"""

import numpy as np
import ml_dtypes

import concourse.bass as bass
import concourse.mybir as mybir
import concourse.tile as tile

F32 = mybir.dt.float32
BF16 = mybir.dt.bfloat16
I16 = mybir.dt.int16

NIMG = 2
H = W = 64
HW = H * W          # 4096
P = 9               # taps
NS = P * HW         # 36864 samples per image
NCHUNK = NS // 128  # 288
NF = NCHUNK * 8     # 2304 idx free size (wrapped-16)
NQ = 4              # SWDGE queues
QROWS = 71 * 71     # quad table rows (5041)

bf = ml_dtypes.bfloat16


# ----------------------------------------------------------------- host prep
def host_consts():
    part = np.arange(128)[:, None]          # [128,1]
    chunk = np.arange(NCHUNK)[None, :]      # [1,288]
    p = chunk // 32                          # tap
    l = (chunk % 32) * 128 + part            # [128,288]
    ho, wo = l // 64, l % 64
    ky, kx = p // 3, p % 3
    ay = (ky + ho - 1).astype(np.float32)
    ax = (kx + wo - 1).astype(np.float32)
    ident = np.eye(128, dtype=bf)
    return dict(ay=ay, ax=ax, ident=ident)


def host_prep_weights(offset_w, offset_b, deform_w, gamma, beta, rm, rv, eps=1e-5):
    # offset_w [18,128,3,3] -> woff [128 cin, 9 tap, 18]
    woff = np.transpose(offset_w, (1, 2, 3, 0)).reshape(128, 9, 18).astype(bf)
    offb = offset_b.reshape(18, 1).astype(np.float32)
    # deform_w [128 o,128 c,3,3] -> wdef [128 c, 9 tap, 128 o]
    wdef = np.transpose(deform_w, (1, 2, 3, 0)).reshape(128, 9, 128).astype(bf)
    scale = (gamma / np.sqrt(rv + eps)).astype(np.float32).reshape(128, 1)
    shift = (beta - rm * gamma / np.sqrt(rv + eps)).astype(np.float32).reshape(128, 1)
    return dict(woff=woff, offb=offb, wdef=wdef, bnscale=scale, bnshift=shift)


def host_prep_images(x2):
    """x2: [n,128,64,64] f32 -> xpad bf16 [n,128,4356], xquad bf16 [n,5041,512].

    xquad row j', j' = (y0+4)*71 + (x0+4) for corner base (y0,x0) in
    [-4,66]^2, holds (X[y0,x0], X[y0,x0+1], X[y0+1,x0], X[y0+1,x0+1])
    channel vectors, zero outside the image.
    """
    n = x2.shape[0]
    xpad = np.zeros((n, 128, 66, 66), dtype=bf)
    xpad[:, :, 1:65, 1:65] = x2.astype(bf)
    xpad = xpad.reshape(n, 128, 66 * 66)

    xloc = np.transpose(x2, (0, 2, 3, 1)).astype(bf)        # [n,64,64,128]
    xe = np.zeros((n, 72, 72, 128), dtype=bf)
    xe[:, 4:68, 4:68] = xloc
    xquad = np.concatenate(
        [xe[:, :71, :71], xe[:, :71, 1:72], xe[:, 1:72, :71], xe[:, 1:72, 1:72]],
        axis=3).reshape(n, QROWS, 512)
    return xpad, np.ascontiguousarray(xquad)


def make_in_map(x2, offset_w, offset_b, deform_w, gamma, beta, rm, rv):
    con = host_consts()
    wts = host_prep_weights(offset_w, offset_b, deform_w, gamma, beta, rm, rv)
    xpad, xquad = host_prep_images(x2)
    return dict(
        xpad=np.ascontiguousarray(xpad), xquad=xquad,
        woff=np.ascontiguousarray(wts["woff"]), offb=wts["offb"],
        wdef=np.ascontiguousarray(wts["wdef"]),
        bnscale=wts["bnscale"], bnshift=wts["bnshift"],
        ay=con["ay"], ax=con["ax"], ident=con["ident"],
    )


# ------------------------------------------------------------------- builder
def build(nc, stage="full", nimg=NIMG):
    dt = nc.dram_tensor
    xpad_d = dt("xpad", [nimg, 128, 66 * 66], BF16, kind="ExternalInput")
    xquad_d = dt("xquad", [nimg, QROWS, 512], BF16, kind="ExternalInput")
    woff_d = dt("woff", [128, 9, 18], BF16, kind="ExternalInput")
    offb_d = dt("offb", [18, 1], F32, kind="ExternalInput")
    wdef_d = dt("wdef", [128, 9, 128], BF16, kind="ExternalInput")
    bnscale_d = dt("bnscale", [128, 1], F32, kind="ExternalInput")
    bnshift_d = dt("bnshift", [128, 1], F32, kind="ExternalInput")
    ay_d = dt("ay", [128, NCHUNK], F32, kind="ExternalInput")
    ax_d = dt("ax", [128, NCHUNK], F32, kind="ExternalInput")
    ident_d = dt("ident", [128, 128], BF16, kind="ExternalInput")

    y_d = dt("y", [nimg, 128, HW], F32, kind="ExternalOutput")
    dbg = {}
    if stage == "off":
        dbg["off"] = dt("dbg_off", [nimg, 18, HW], F32, kind="ExternalOutput")
    if stage == "idx":
        dbg["omt"] = dt("dbg_omt", [nimg, 128, NCHUNK, 2], BF16, kind="ExternalOutput")
        dbg["omb"] = dt("dbg_omb", [nimg, 128, NCHUNK, 2], BF16, kind="ExternalOutput")
        dbg["idxt"] = dt("dbg_idxt", [nimg, 128, NF], I16, kind="ExternalOutput")
    if stage in ("gather", "blend"):
        dbg["st"] = dt("dbg_st", [nimg, 128, NS], BF16, kind="ExternalOutput")

    ALU = mybir.AluOpType
    ACT = mybir.ActivationFunctionType

    with tile.TileContext(nc) as tc:
        with (
            tc.tile_pool(name="const", bufs=1) as cpool,
            tc.tile_pool(name="xin", bufs=1) as xpool,
            tc.tile_pool(name="offp", bufs=1) as offpool,
            tc.tile_pool(name="idxp", bufs=1) as idxpool,
            tc.tile_pool(name="gat", bufs=1) as gpool,
            tc.tile_pool(name="sall", bufs=1) as spool,
            tc.tile_pool(name="yout", bufs=2) as ypool,
            tc.tile_pool(name="psoff", bufs=2, space="PSUM") as psoff,
            tc.tile_pool(name="pstr", bufs=3, space="PSUM") as pstr,
            tc.tile_pool(name="psy", bufs=2, space="PSUM") as psy,
            tc.tile_pool(name="dram", bufs=2, space="DRAM") as dpool,
        ):
            C = {}
            for nm, d, shp, ddt in [
                ("woff", woff_d, [128, 9, 18], BF16),
                ("offb", offb_d, [18, 1], F32),
                ("wdef", wdef_d, [128, 9, 128], BF16),
                ("bnscale", bnscale_d, [128, 1], F32),
                ("bnshift", bnshift_d, [128, 1], F32),
                ("ay", ay_d, [128, NCHUNK], F32),
                ("ax", ax_d, [128, NCHUNK], F32),
                ("ident", ident_d, [128, 128], BF16),
            ]:
                s = cpool.tile(shp, ddt, name=nm, tag=nm)
                nc.sync.dma_start(s[:], d[:])
                C[nm] = s

            for img in range(nimg):
                _image(nc, tc, stage, img, dbg, xpad_d, xquad_d, y_d, C,
                       xpool, offpool, idxpool, gpool, spool, ypool,
                       psoff, pstr, psy, dpool, ALU, ACT)
    return nc


def _image(nc, tc, stage, img, dbg, xpad_d, xquad_d, y_d, C,
           xpool, offpool, idxpool, gpool, spool, ypool,
           psoff, pstr, psy, dpool, ALU, ACT):
    # ---- A. offset conv
    xpad_s = xpool.tile([128, 66 * 66], BF16, name="xpad", tag="xpad")
    nc.sync.dma_start(xpad_s[:], xpad_d[img])
    xv = xpad_s[:].rearrange("c (h w) -> c h w", h=66)

    off_s = offpool.tile([18, HW], F32, name="off", tag="off")
    for q in range(8):
        ps = psoff.tile([18, 512], F32, name="offps", tag="offps")
        for p in range(9):
            ky, kx = p // 3, p % 3
            rhs = xv[:, 8 * q + ky: 8 * q + ky + 8, kx: kx + 64]
            nc.tensor.matmul(ps[:], C["woff"][:, p, :], rhs,
                             start=(p == 0), stop=(p == 8))
        nc.vector.tensor_scalar(
            off_s[:, 512 * q: 512 * (q + 1)], ps[:], C["offb"][:], None, ALU.add
        )
    if stage == "off":
        nc.sync.dma_start(dbg["off"][img], off_s[:])
        return

    # ---- B. wrap-read dy/dx
    offd = dpool.tile([18, HW], F32, name="offd", tag=f"offd{img}")
    nc.sync.dma_start(offd[:], off_s[:])
    dy_s = idxpool.tile([128, NCHUNK], F32, name="dy", tag="dy")
    dx_s = idxpool.tile([128, NCHUNK], F32, name="dx", tag="dx")
    offd_t = offd[:].tensor
    for p in range(9):
        nc.sync.dma_start(
            dy_s[:, 32 * p: 32 * (p + 1)],
            bass.AP(offd_t, 2 * p * HW, [[1, 128], [128, 32]]),
        )
        nc.sync.dma_start(
            dx_s[:, 32 * p: 32 * (p + 1)],
            bass.AP(offd_t, (2 * p + 1) * HW, [[1, 128], [128, 32]]),
        )

    # ---- C. index math (zero-padded quad table: no validity masks)
    def t(tag):
        return idxpool.tile([128, NCHUNK], F32, name=tag, tag=tag)

    I32 = mybir.dt.int32
    py = t("py"); nc.vector.tensor_tensor(py[:], dy_s[:], C["ay"][:], ALU.add)
    yi = idxpool.tile([128, NCHUNK], I32, name="i32y", tag="i32y")
    nc.vector.tensor_copy(yi[:], py[:])
    yf = t("yf");  nc.vector.tensor_copy(yf[:], yi[:])
    ygt = t("ygt"); nc.vector.tensor_tensor(ygt[:], yf[:], py[:], ALU.is_gt)
    y0 = t("y0");  nc.vector.tensor_tensor(y0[:], yf[:], ygt[:], ALU.subtract)
    wy = t("wy");  nc.vector.tensor_tensor(wy[:], py[:], y0[:], ALU.subtract)
    y0c = t("y0c"); nc.vector.tensor_scalar(y0c[:], y0[:], -4.0, 66.0, ALU.max, ALU.min)

    px = idxpool.tile([128, NCHUNK], F32, name="px", tag="py")
    nc.vector.tensor_tensor(px[:], dx_s[:], C["ax"][:], ALU.add)
    xi = idxpool.tile([128, NCHUNK], I32, name="i32x", tag="i32y")
    nc.vector.tensor_copy(xi[:], px[:])
    xf = idxpool.tile([128, NCHUNK], F32, name="xf", tag="yf")
    nc.vector.tensor_copy(xf[:], xi[:])
    xgt = idxpool.tile([128, NCHUNK], F32, name="xgt", tag="ygt")
    nc.vector.tensor_tensor(xgt[:], xf[:], px[:], ALU.is_gt)
    x0 = t("x0");  nc.vector.tensor_tensor(x0[:], xf[:], xgt[:], ALU.subtract)
    wx = t("wx");  nc.vector.tensor_tensor(wx[:], px[:], x0[:], ALU.subtract)
    x0c = t("x0c"); nc.vector.tensor_scalar(x0c[:], x0[:], -4.0, 66.0, ALU.max, ALU.min)

    w1y = t("w1y"); nc.vector.tensor_scalar(w1y[:], wy[:], -1.0, 1.0, ALU.mult, ALU.add)
    w1x = t("w1x"); nc.vector.tensor_scalar(w1x[:], wx[:], -1.0, 1.0, ALU.mult, ALU.add)

    omt = idxpool.tile([128, NCHUNK, 2], BF16, name="omt", tag=f"omt{img}")
    omb = idxpool.tile([128, NCHUNK, 2], BF16, name="omb", tag=f"omb{img}")
    nc.vector.tensor_tensor(omt[:, :, 0], w1y[:], w1x[:], ALU.mult)
    nc.vector.tensor_tensor(omt[:, :, 1], w1y[:], wx[:], ALU.mult)
    nc.vector.tensor_tensor(omb[:, :, 0], wy[:], w1x[:], ALU.mult)
    nc.vector.tensor_tensor(omb[:, :, 1], wy[:], wx[:], ALU.mult)

    # j = (y0+4)*71 + (x0+4) = 71*y0 + x0 + 288
    jtf = t("jtf")
    nc.vector.scalar_tensor_tensor(jtf[:], y0c[:], 71.0, x0c[:], ALU.mult, ALU.add)
    nc.vector.tensor_scalar(jtf[:], jtf[:], 288.0, None, ALU.add)
    jt16 = idxpool.tile([128, NCHUNK], I16, name="jt16", tag="jt16")
    nc.vector.tensor_copy(jt16[:], jtf[:])

    # ---- D. wrap-16 replicated idx tensor
    jd = dpool.tile([128, NCHUNK], I16, name="jd", tag=f"jd{img}")
    nc.sync.dma_start(jd[:], jt16[:])
    idxt = idxpool.tile([128, NF], I16, name="idxt", tag=f"idxt{img}")
    jd_t = jd[:].tensor
    # dst column f = 8*chunk + g holds j[p16 + 16g, chunk], replicated over
    # the 8 16-partition groups. One strided DMA per g.
    for g in range(8):
        src = [[0, 8], [NCHUNK, 16], [1, NCHUNK]]
        nc.sync.dma_start(
            bass.AP(idxt[:].tensor, g, [[NF, 128], [8, NCHUNK]]),
            bass.AP(jd_t, 16 * g * NCHUNK, [list(r) for r in src]),
        )

    if stage == "idx":
        nc.sync.dma_start(dbg["omt"][img], omt[:])
        nc.sync.dma_start(dbg["omb"][img], omb[:])
        nc.sync.dma_start(dbg["idxt"][img], idxt[:])
        return

    # ---- E/F/G. gather + blend + transpose, per group of NG chunks
    s_all = spool.tile([128, 9, HW], BF16, name="sall", tag="sall")
    NG = 8                                    # chunks per gather group
    NGRP = NCHUNK // NG                       # 36 groups
    NBUF = 4
    nq = getattr(nc, "num_swdge_queues", 1)
    for g in range(NGRP):
        c0 = NG * g                           # first global chunk of group
        nid = NG * 128
        gt4 = gpool.tile([128, NG, 512], BF16, name="gt4", tag=f"gt{g % NBUF}")
        f0 = 8 * c0
        nc.gpsimd.dma_gather(
            gt4[:], xquad_d[img], idxt[:, f0: f0 + NG * 8],
            num_idxs=nid, num_idxs_reg=nid, elem_size=512,
            queue_num=g % nq,
        )
        m0 = gpool.tile([128, NG, 128], BF16, name="m0", tag="m0")
        m1 = gpool.tile([128, NG, 128], BF16, name="m1", tag="m1")
        m2 = gpool.tile([128, NG, 128], BF16, name="m2", tag="m2")
        m3 = gpool.tile([128, NG, 128], BF16, name="m3", tag="m3")
        sl = slice(c0, c0 + NG)
        bc = [128, NG, 128]
        nc.vector.tensor_tensor(m0[:], gt4[:, :, 0:128], omt[:, sl, 0].unsqueeze(2).broadcast_to(bc), ALU.mult)
        nc.vector.tensor_tensor(m1[:], gt4[:, :, 128:256], omt[:, sl, 1].unsqueeze(2).broadcast_to(bc), ALU.mult)
        nc.vector.tensor_tensor(m2[:], gt4[:, :, 256:384], omb[:, sl, 0].unsqueeze(2).broadcast_to(bc), ALU.mult)
        nc.vector.tensor_tensor(m3[:], gt4[:, :, 384:512], omb[:, sl, 1].unsqueeze(2).broadcast_to(bc), ALU.mult)
        st = gpool.tile([128, NG, 128], BF16, name="stt", tag="stt")
        nc.vector.tensor_tensor(m0[:], m0[:], m1[:], ALU.add)
        nc.vector.tensor_tensor(m2[:], m2[:], m3[:], ALU.add)
        nc.vector.tensor_tensor(st[:], m0[:], m2[:], ALU.add)
        if stage in ("gather", "blend"):
            nc.sync.dma_start(
                dbg["st"][img][:, 128 * c0: 128 * c0 + nid],
                st[:].rearrange("a b c -> a (b c)"),
            )
        if stage == "blend":
            continue
        # transpose NG blocks [s,c] -> [c,s]; pack 4 per psum tile, 1 ACT copy
        # group g covers chunks c0..c0+7 of tap p = c0//32; l-block within tap
        # lblk = (c0%32) + tb, contiguous => s_all cols 128*(c0%32) .. +1024
        p = c0 // 32
        for half in range(2):
            tps = pstr.tile([128, 4, 128], BF16, name="tps", tag="tps")
            for j in range(4):
                tb = 4 * half + j
                nc.tensor.transpose(tps[:, j, :], st[:, tb, :], C["ident"][:])
            lc0 = 128 * (c0 % 32) + 512 * half
            nc.scalar.copy(s_all[:, p, lc0: lc0 + 512],
                           tps[:].rearrange("a b c -> a (b c)"))
    if stage in ("gather", "blend"):
        return

    # ---- H. main conv + BN + SiLU
    for q in range(8):
        ps = psy.tile([128, 512], F32, name="yps", tag="yps")
        for p in range(9):
            rhs = s_all[:, p, 512 * q: 512 * (q + 1)]
            nc.tensor.matmul(ps[:], C["wdef"][:, p, :], rhs,
                             start=(p == 0), stop=(p == 8))
        ysb = ypool.tile([128, 512], F32, name="ysb", tag="ysb")
        zt = ypool.tile([128, 512], F32, name="zt", tag="zt")
        sg = ypool.tile([128, 512], F32, name="sg", tag="sg")
        nc.scalar.activation(zt[:], ps[:], ACT.Identity,
                             bias=C["bnshift"][:], scale=C["bnscale"][:])
        nc.scalar.activation(sg[:], ps[:], ACT.Sigmoid,
                             bias=C["bnshift"][:], scale=C["bnscale"][:])
        nc.vector.tensor_tensor(ysb[:], zt[:], sg[:], ALU.mult)
        nc.sync.dma_start(y_d[img][:, 512 * q: 512 * (q + 1)], ysb[:])


# ------------------------------------------------------------ numpy model
def numpy_stages(x2, offset_w, offset_b, deform_w, gamma, beta, rm, rv):
    n = x2.shape[0]
    con = host_consts()
    wts = host_prep_weights(offset_w, offset_b, deform_w, gamma, beta, rm, rv)
    xpad, xquad = host_prep_images(x2)
    out = {}
    xp = xpad.reshape(n, 128, 66, 66).astype(np.float32)
    woff = wts["woff"].astype(np.float32)           # [128,9,18]
    off = np.zeros((n, 18, 64, 64), np.float32)
    for p in range(9):
        ky, kx = p // 3, p % 3
        patch = xp[:, :, ky: ky + 64, kx: kx + 64]
        off += np.einsum("nchw,co->nohw", patch, woff[:, p, :])
    off += wts["offb"].reshape(1, 18, 1, 1)
    out["off"] = off.reshape(n, 18, HW)

    part = np.arange(128)[:, None]
    chunk = np.arange(NCHUNK)[None, :]
    p_ = np.broadcast_to(chunk // 32, (128, NCHUNK))
    l_ = (chunk % 32) * 128 + part
    dy = out["off"][:, 2 * p_, l_]                   # [n,128,288]
    dx = out["off"][:, 2 * p_ + 1, l_]
    py = dy + con["ay"]
    px = dx + con["ax"]
    wy = np.mod(py, 1.0); y0 = py - wy
    wx = np.mod(px, 1.0); x0 = px - wx
    y0c = np.clip(y0, -4, 66)
    x0c = np.clip(x0, -4, 66)
    omt = np.stack([(1 - wy) * (1 - wx), (1 - wy) * wx], axis=-1).astype(bf)
    omb = np.stack([wy * (1 - wx), wy * wx], axis=-1).astype(bf)
    jt = (71 * y0c + x0c + 288).astype(np.int16)
    out.update(omt=omt, omb=omb, jt=jt)

    st = np.zeros((n, 128, NS), dtype=bf)
    for i in range(n):
        g4 = xquad[i][jt[i]]            # [128,288,512] bf16
        omt_b = np.repeat(omt[i][:, :, :, None], 128, axis=3).reshape(128, NCHUNK, 256)
        omb_b = np.repeat(omb[i][:, :, :, None], 128, axis=3).reshape(128, NCHUNK, 256)
        a = (g4[:, :, 0:256] * omt_b).astype(bf)
        b = (g4[:, :, 256:512] * omb_b).astype(bf)
        s1 = (a[:, :, 0:128] + a[:, :, 128:256]).astype(bf)
        s2 = (b[:, :, 0:128] + b[:, :, 128:256]).astype(bf)
        st[i] = (s1 + s2).astype(bf).reshape(128, NS)
    out["st"] = st

    y = np.zeros((n, 128, HW), np.float32)
    wdef = wts["wdef"].astype(np.float32)           # [128,9,128]
    for i in range(n):
        s_sm = st[i].astype(np.float32).reshape(128, NCHUNK, 128)
        s_cs = np.transpose(s_sm, (2, 1, 0)).reshape(128, NS)  # [c, s]
        for p in range(9):
            sl = s_cs[:, p * HW: (p + 1) * HW]
            y[i] += np.einsum("cl,co->ol", sl, wdef[:, p, :])
    scale = wts["bnscale"].reshape(1, 128, 1)
    shift = wts["bnshift"].reshape(1, 128, 1)
    ybn = y * scale + shift
    out["y"] = ybn * (1.0 / (1.0 + np.exp(-ybn)))
    return out


# ============================= tilefix =============================
from concourse.vector_clock import ScopedClock

_MAX_WAITS = 1


def _patched_drain_and_barrier(self, tick_clock, wait_clock):
    nc = self.nc
    collector = nc.sync.nop(nofuse=True)
    wait_clock.add_sem_waits(
        collector.ins, ScopedClock({None: tick_clock.global_clock})
    )
    si = collector.ins.sync_info
    waits = list(si.on_wait or []) if si is not None else []
    if si is not None:
        si.on_wait = waits[:_MAX_WAITS]
    for i in range(_MAX_WAITS, len(waits), _MAX_WAITS):
        n = nc.sync.nop(nofuse=True)
        nsi = n.ins.sync_info
        if nsi is None:
            n.ins.sync_info = type(si)(on_wait=waits[i : i + _MAX_WAITS], on_update=[])
        else:
            nsi.on_wait = waits[i : i + _MAX_WAITS]
    nc.sync.drain()

    nc.all_engine_barrier()
    assert self.sems is not None
    popped = nc._tile_sem_poison_stack.pop()
    assert popped is self._sem_poison
    nc.clear_and_free_semaphores(list(self.sems.allocated().values()))
    nc.all_engine_barrier()


def _apply_tilefix():
    tile.TileContext._drain_and_barrier = _patched_drain_and_barrier


# ===================================================================== kernel
_NC_CACHE = {}


def _build_nc():
    if "nc" not in _NC_CACHE:
        _apply_tilefix()
        import concourse.bacc as bacc
        nc = bacc.Bacc(None, num_swdge_queues=NQ)
        build(nc, stage="full", nimg=NIMG)
        nc.compile()
        _NC_CACHE["nc"] = nc
    return _NC_CACHE["nc"]


def kernel(x, offset_w, offset_b, deform_w, gamma, beta, running_mean,
           running_var):
    from concourse.bass_utils import run_bass_kernel_spmd

    x = np.asarray(x); offset_w = np.asarray(offset_w)
    offset_b = np.asarray(offset_b); deform_w = np.asarray(deform_w)
    gamma = np.asarray(gamma); beta = np.asarray(beta)
    rm = np.asarray(running_mean); rv = np.asarray(running_var)

    nc = _build_nc()
    n_cores = 8
    per = x.shape[0] // n_cores  # 2
    in_maps = []
    for i in range(n_cores):
        in_maps.append(make_in_map(
            x[per * i: per * (i + 1)], offset_w, offset_b, deform_w,
            gamma, beta, rm, rv))
    res = run_bass_kernel_spmd(nc, in_maps, list(range(n_cores)))
    out = np.concatenate([r["y"] for r in res.results], axis=0)
    return out.reshape(16, 128, 64, 64).astype(np.float32)
